# revision 18
# baseline (speedup 1.0000x reference)
"""TRN2 Bass kernel for nn_CombinedLossWithEMD (chamfer + repulsion +
smoothness + coverage point-cloud loss).

Distribution: 8 cores, SPMD. Core c handles batch b = c//2, row-half
h = c%2. Every pairwise-distance matrix is computed row-blocked: the
"query" side is the core's half (2048 pred / 2048 gt / 1024 partial
rows), the "database" side is a full 4096-point cloud.

Distances: PE matmul with K=24 augmented vectors produces NEGATED
squared distances (-d^2 = 2q.b - |q|^2 - |b|^2) directly in PSUM. All
operands are bf16 hi/mid/lo triplets, so every product is exact in the
fp32 PSUM accumulation: d^2 comes out with ~fp32 accuracy at bf16
matmul speed (1 PE cycle/row vs 4 for fp32).

Mins (chamfer/coverage): DVE tensor_reduce(max) straight off PSUM.
Top-16 (repulsion/smoothness): ACT copies -d^2 to SBUF, then DVE
max8 -> match_replace -> max8 gives the exact 16 largest -d^2 (= 16
nearest) per row. Self-distance is forced to the reference's
sqrt(EPS)=1e-6 by overwriting rank 0.

Each core returns [128, 5] fp32 per-partition partial sums; the host
reduces and assembles the 5 scalar outputs.
"""
import numpy as np
import ml_dtypes
from contextlib import ExitStack

BF = ml_dtypes.bfloat16

B = 4
N = 4096          # pred/gt points per batch
KP = 2048         # partial points per batch
NCORES = 8
HALF_N = N // 2   # 2048 query rows per core (pred/gt)
HALF_P = KP // 2  # 1024 partial query rows per core
KAUG = 24

CHAMFER_W, REP_W, SMOOTH_W, COV_W = 1.0, 0.01, 0.005, 0.1
REP_THRESHOLD = 0.01

_NC_CACHE = {}


def _split3(x):
    h = x.astype(BF).astype(np.float32)
    m = (x - h).astype(BF).astype(np.float32)
    l = (x - h - m).astype(BF).astype(np.float32)
    return h, m, l


def _aug_query(q):
    """q [n,3] fp32 -> [24, n] bf16 lhsT rows (query side, negated norms)."""
    n = q.shape[0]
    qh, qm, ql = _split3(q)
    nq = (q * q).sum(-1)
    nqh, nqm, nql = _split3(nq)
    rows = np.zeros((KAUG, n), np.float32)
    rows[0:3] = 2 * qh.T
    rows[3:6] = 2 * qh.T
    rows[6:9] = 2 * qm.T
    rows[9:12] = 2 * qh.T
    rows[12:15] = 2 * ql.T
    rows[15:18] = 2 * qm.T
    rows[18] = -nqh
    rows[19] = -nqm
    rows[20] = -nql
    rows[21] = -1.0
    rows[22] = -1.0
    rows[23] = -1.0
    return np.ascontiguousarray(rows.astype(BF))


def _aug_db(b):
    """b [m,3] fp32 -> [24, m] bf16 rhs rows (database side)."""
    m_ = b.shape[0]
    bh, bm, bl = _split3(b)
    nb = (b * b).sum(-1)
    nbh, nbm, nbl = _split3(nb)
    rows = np.zeros((KAUG, m_), np.float32)
    rows[0:3] = bh.T
    rows[3:6] = bm.T
    rows[6:9] = bh.T
    rows[9:12] = bl.T
    rows[12:15] = bh.T
    rows[15:18] = bm.T
    rows[18] = 1.0
    rows[19] = 1.0
    rows[20] = 1.0
    rows[21] = nbh
    rows[22] = nbm
    rows[23] = nbl
    return np.ascontiguousarray(rows.astype(BF))


TOPK_FP32 = True


def _build_nc(repeat=1, n_gp=0, topk_fp32=TOPK_FP32):
    """repeat>1 wraps the body in a timing loop (benchmarking only)."""
    import concourse.bacc as bacc
    import concourse.mybir as mybir
    import concourse.tile as tile

    FP32 = mybir.dt.float32
    BF16 = mybir.dt.bfloat16
    TK = FP32 if topk_fp32 else BF16
    AX = mybir.AxisListType.X
    OP = mybir.AluOpType
    ACTF = mybir.ActivationFunctionType

    nc = bacc.Bacc("TRN2", target_bir_lowering=False, debug=False)

    qp = nc.dram_tensor("qp", [KAUG, HALF_N], BF16, kind="ExternalInput").ap()
    qg = nc.dram_tensor("qg", [KAUG, HALF_N], BF16, kind="ExternalInput").ap()
    qc = nc.dram_tensor("qc", [KAUG, HALF_P], BF16, kind="ExternalInput").ap()
    dg = nc.dram_tensor("dg", [KAUG, N], BF16, kind="ExternalInput").ap()
    dp = nc.dram_tensor("dp", [KAUG, N], BF16, kind="ExternalInput").ap()
    out = nc.dram_tensor("out", [128, 5], FP32, kind="ExternalOutput").ap()

    NT_A = HALF_N // 128   # 16 row-tiles for A (pred half vs gt)
    NT_B = HALF_N // 128   # 16 for B (gt half vs pred)
    NT_C = HALF_P // 128   # 8 for C (partial half vs pred)
    NT_D = HALF_N // 128   # 16 for D (pred half vs pred, top-16)

    with tile.TileContext(nc) as tc, ExitStack() as ctx:
        const = ctx.enter_context(tc.tile_pool(name="const", bufs=1))
        work = ctx.enter_context(tc.tile_pool(name="work", bufs=4))
        ps = ctx.enter_context(tc.tile_pool(name="ps", bufs=2, space="PSUM"))

        dps = const.tile([KAUG, N], BF16)
        qps = const.tile([KAUG, HALF_N], BF16)
        dgs = const.tile([KAUG, N], BF16)
        qgs = const.tile([KAUG, HALF_N], BF16)
        qcs = const.tile([KAUG, HALF_P], BF16)

        def load_inputs():
            # split the critical dp/qp loads across both HWDGE queues and
            # SWDGE so the first D/B/C matmuls start sooner
            nc.sync.dma_start(dps[:, 0:N // 2], dp[:, 0:N // 2])
            nc.scalar.dma_start(dps[:, N // 2:N], dp[:, N // 2:N])
            nc.gpsimd.dma_start(qps[:], qp)
            nc.sync.dma_start(dgs[:], dg)
            nc.scalar.dma_start(qgs[:], qg)
            nc.gpsimd.dma_start(qcs[:], qc)

        # chamfer maxes of -d^2: 2 columns per row-tile (one per psum half)
        mA = const.tile([128, 2 * NT_A], FP32)
        mB = const.tile([128, 2 * NT_B], FP32)
        mC = const.tile([128, 2 * NT_C], FP32)
        thr = const.tile([128, 1], FP32)      # repulsion threshold bias
        nc.gpsimd.memset(thr[:], float(REP_THRESHOLD))
        # D per-row-tile stats
        s1c = const.tile([128, NT_D], FP32)   # sum of 16 NN distances
        s2n = const.tile([128, NT_D], FP32)   # sum of -d^2 over 16 NN
        repc = const.tile([128, NT_D], FP32)  # sum relu(thr - d_{1..4})
        v16all = const.tile([128, 16 * NT_D], TK)  # top-16 -d^2 per row-tile
        S = const.tile([128, 5], FP32)        # final per-partition sums

        def mm_rowtile(q_sb, db_sb, t, consume, key):
            """8 matmuls of row-tile t (queries q_sb[:, t*128:...]) against
            db_sb, in two 4-bank PSUM tiles; consume(half, pt) eats each."""
            lhsT = q_sb[:, t * 128:(t + 1) * 128]
            for half in range(2):
                pt = ps.tile([128, 2048], FP32, tag="pt", name=f"pt_{key}_{half}")
                for j in range(4):
                    col = half * 4 + j
                    nc.tensor.matmul(
                        pt[:, j * 512:(j + 1) * 512],
                        lhsT,
                        db_sb[:, col * 512:(col + 1) * 512],
                        start=True, stop=True,
                    )
                consume(half, pt)

        def chamfer_dve(q_sb, db_sb, t, mdst, key):
            """row mins via DVE tensor_reduce straight off PSUM."""
            def consume(half, pt):
                nc.vector.tensor_reduce(
                    mdst[:, 2 * t + half:2 * t + half + 1], pt[:],
                    axis=AX, op=OP.max)
            mm_rowtile(q_sb, db_sb, t, consume, key)

        def self_rowtile(t):
            dt_ = work.tile([128, N], TK, tag="dt", name=f"dt_{t}")

            def consume(half, pt):
                nc.scalar.activation(
                    dt_[:, half * 2048:(half + 1) * 2048], pt[:], ACTF.Copy)

            mm_rowtile(qps, dps, t, consume, f"D{t}")

            v16 = v16all[:, 16 * t:16 * (t + 1)]
            nc.vector.max(v16[:, 0:8], dt_[:])
            nc.vector.match_replace(dt_[:], v16[:, 0:8], dt_[:], -1e30)
            nc.vector.max(v16[:, 8:16], dt_[:])

        # chamfer work list: C first (offload candidates), then A, then B
        chamfer_jobs = (
            [(qcs, dps, t, mC, f"C{t}") for t in range(NT_C)]
            + [(qps, dgs, t, mA, f"A{t}") for t in range(NT_A)]
            + [(qgs, dps, t, mB, f"B{t}") for t in range(NT_B)]
        )
        routed = [(chamfer_dve, j) for j in chamfer_jobs]

        def body():
            # interleave D with chamfer jobs so PE/ACT/GPSIMD overlap the
            # DVE-bound top-16 work
            jobs_per_d = (len(routed) + NT_D - 1) // NT_D
            k = 0
            for t in range(NT_D):
                self_rowtile(t)
                for fn, j in routed[k:k + jobs_per_d]:
                    fn(*j)
                k += jobs_per_d

            # finish: clamp then sqrt of chamfer maxes with accumulation;
            # pair-combine the 2 psum-half columns per row-tile first
            mAf = work.tile([128, NT_A], FP32, tag="mAf")
            nc.vector.tensor_reduce(
                mAf[:], mA.rearrange("p (t two) -> p t two", two=2),
                axis=AX, op=OP.max)
            mBf = work.tile([128, NT_B], FP32, tag="mBf")
            nc.vector.tensor_reduce(
                mBf[:], mB.rearrange("p (t two) -> p t two", two=2),
                axis=AX, op=OP.max)
            mCf = work.tile([128, NT_C], FP32, tag="mCf")
            nc.vector.tensor_reduce(
                mCf[:], mC.rearrange("p (t two) -> p t two", two=2),
                axis=AX, op=OP.max)
            nc.gpsimd.tensor_scalar_min(mAf[:], mAf[:], -1e-12)
            nc.gpsimd.tensor_scalar_min(mBf[:], mBf[:], -1e-12)
            nc.gpsimd.tensor_scalar_min(mCf[:], mCf[:], -1e-12)
            dumA = work.tile([128, NT_A], FP32, tag="dumA")
            nc.scalar.activation(
                dumA[:], mAf[:], ACTF.Sqrt, scale=-1.0, accum_out=S[:, 0:1])
            dumB = work.tile([128, NT_B], FP32, tag="dumB")
            nc.scalar.activation(
                dumB[:], mBf[:], ACTF.Sqrt, scale=-1.0, accum_out=S[:, 1:2])
            dumC = work.tile([128, NT_C], FP32, tag="dumC")
            nc.scalar.activation(
                dumC[:], mCf[:], ACTF.Sqrt, scale=-1.0, accum_out=S[:, 2:3])

            # batched D finishing over all row-tiles' top-16 values:
            # force self-distance to the reference's sqrt(EPS), clamp
            # -d^2 <= -EPS (matches reference max(sq, EPS); also kills
            # positive fp noise on ultra-close pairs -> no sqrt NaN)
            v16v = v16all.rearrange("p (t k) -> p t k", k=16)
            nc.gpsimd.memset(v16v[:, :, 0:1], -1e-12)
            nc.gpsimd.tensor_scalar_min(v16all[:], v16all[:], -1e-12)
            nc.vector.tensor_reduce(s2n[:], v16v, axis=AX, op=OP.add)
            d16all = work.tile([128, 16 * NT_D], FP32, tag="d16all")
            nc.scalar.activation(
                d16all[:], v16all[:], ACTF.Sqrt, scale=-1.0)
            d16v = d16all.rearrange("p (t k) -> p t k", k=16)
            nc.vector.tensor_reduce(s1c[:], d16v, axis=AX, op=OP.add)
            rep4all = work.tile([128, 4 * NT_D], FP32, tag="rep4all")
            nc.scalar.activation(
                rep4all[:], d16v[:, :, 1:5], ACTF.Relu, scale=-1.0,
                bias=thr[:])
            nc.vector.tensor_reduce(
                repc[:], rep4all.rearrange("p (t k) -> p t k", k=4),
                axis=AX, op=OP.add)

            # 15*var per row-tile: -s2n - s1^2/16 ; accum over row-tiles
            t1 = work.tile([128, NT_D], FP32, tag="t1")
            nc.gpsimd.tensor_tensor(t1[:], s1c[:], s1c[:], op=OP.mult)
            var15 = work.tile([128, NT_D], FP32, tag="var15")
            nc.vector.scalar_tensor_tensor(
                var15[:], t1[:], -1.0 / 16.0, s2n[:],
                op0=OP.mult, op1=OP.subtract,
                accum_out=S[:, 3:4])
            nc.vector.tensor_reduce(S[:, 4:5], repc[:], axis=AX, op=OP.add)

        if repeat == 1:
            load_inputs()
            body()
        else:
            # input DMAs live inside the loop so no dependency crosses the
            # back-edge semaphore reset
            with tc.For_i(0, repeat, 1):
                load_inputs()
                body()

        nc.gpsimd.dma_start(out, S[:])

    nc.compile()
    return nc


def _get_nc():
    if "nc" not in _NC_CACHE:
        _NC_CACHE["nc"] = _build_nc()
    return _NC_CACHE["nc"]


def _make_in_maps(pred, gt, partial):
    in_maps = []
    dbg = [_aug_db(gt[b]) for b in range(B)]
    dbp = [_aug_db(pred[b]) for b in range(B)]
    for c in range(NCORES):
        b, h = divmod(c, 2)
        in_maps.append({
            "qp": _aug_query(pred[b, h * HALF_N:(h + 1) * HALF_N]),
            "qg": _aug_query(gt[b, h * HALF_N:(h + 1) * HALF_N]),
            "qc": _aug_query(partial[b, h * HALF_P:(h + 1) * HALF_P]),
            "dg": dbg[b],
            "dp": dbp[b],
        })
    return in_maps


def _combine(results):
    S = np.stack([r["out"] for r in results]).astype(np.float64)  # [8,128,5]
    tot = S.sum(axis=(0, 1))
    cd = tot[0] / (B * N) + tot[1] / (B * N)
    cov = tot[2] / (B * KP)
    smooth = tot[3] / 15.0 / (B * N)
    rep = tot[4] / (B * N * 4)
    total = (CHAMFER_W * cd + REP_W * rep + SMOOTH_W * smooth + COV_W * cov)
    return tuple(np.float32(x) for x in (total, cd, rep, smooth, cov))


def _get_runner():
    """Cached jitted SPMD executor (mirrors bass2jax.run_bass_via_pjrt but
    reuses the traced/jitted callable across kernel() calls)."""
    if "runner" in _NC_CACHE:
        return _NC_CACHE["runner"]
    import jax
    import concourse.mybir as mybir
    from concourse import bass2jax
    from jax.experimental.shard_map import shard_map
    from jax.sharding import Mesh, PartitionSpec

    nc = _get_nc()
    bass2jax.install_neuronx_cc_hook()
    assert nc.dbg_addr is None
    pname = nc.partition_id_tensor.name if nc.partition_id_tensor else None

    in_names, out_names, out_avals, zero_outs = [], [], [], []
    for alloc in nc.m.functions[0].allocations:
        if not isinstance(alloc, mybir.MemoryLocationSet):
            continue
        name = alloc.memorylocations[0].name
        if alloc.kind == "ExternalInput":
            if name != pname:
                in_names.append(name)
        elif alloc.kind == "ExternalOutput":
            shape = tuple(alloc.tensor_shape)
            dtype = mybir.dt.np(alloc.dtype)
            out_names.append(name)
            out_avals.append(jax.core.ShapedArray(shape, dtype))
            zero_outs.append(np.zeros((NCORES * shape[0], *shape[1:]), dtype))
    n_params = len(in_names)
    all_in_names = in_names + out_names
    if pname is not None:
        all_in_names = all_in_names + [pname]
    donate = tuple(range(n_params, n_params + len(out_names)))

    def _body(*args):
        operands = list(args)
        if pname is not None:
            operands.append(bass2jax.partition_id_tensor())
        outs = bass2jax._bass_exec_p.bind(
            *operands,
            out_avals=tuple(out_avals),
            in_names=tuple(all_in_names),
            out_names=tuple(out_names),
            lowering_input_output_aliases=(),
            sim_require_finite=True,
            sim_require_nnan=True,
            nc=nc,
        )
        return tuple(outs)

    devices = jax.devices()[:NCORES]
    mesh = Mesh(np.asarray(devices), ("core",))
    nio = n_params + len(out_names)
    sharded = jax.jit(
        shard_map(
            _body, mesh=mesh,
            in_specs=(PartitionSpec("core"),) * nio,
            out_specs=(PartitionSpec("core"),) * len(out_names),
            check_rep=False,
        ),
        donate_argnums=donate,
        keep_unused=True,
    )

    def run(in_maps):
        concat_in = [
            np.concatenate([m[name] for m in in_maps], axis=0)
            for name in in_names
        ]
        out_arrs = sharded(*concat_in, *[z.copy() for z in zero_outs])
        return [
            {
                name: np.asarray(out_arrs[i]).reshape(
                    NCORES, *out_avals[i].shape)[c]
                for i, name in enumerate(out_names)
            }
            for c in range(NCORES)
        ]

    _NC_CACHE["runner"] = run
    return run


def kernel(pred, gt, partial):
    pred = np.asarray(pred, dtype=np.float32)
    gt = np.asarray(gt, dtype=np.float32)
    partial = np.asarray(partial, dtype=np.float32)

    run = _get_runner()
    in_maps = _make_in_maps(pred, gt, partial)
    return _combine(run(in_maps))


# revision 23
# speedup vs baseline: 1.0729x; 1.0729x over previous
"""TRN2 Bass kernel for nn_CombinedLossWithEMD (chamfer + repulsion +
smoothness + coverage point-cloud loss).

Distribution: 8 cores, SPMD. Core c handles batch b = c//2, row-half
h = c%2. Every pairwise-distance matrix is computed row-blocked: the
"query" side is the core's half (2048 pred / 2048 gt / 1024 partial
rows), the "database" side is a full 4096-point cloud.

Distances: PE matmul with K=24 augmented vectors produces NEGATED
squared distances (-d^2 = 2q.b - |q|^2 - |b|^2) directly in PSUM. All
operands are bf16 hi/mid/lo triplets, so every product is exact in the
fp32 PSUM accumulation: d^2 comes out with ~fp32 accuracy at bf16
matmul speed (1 PE cycle/row vs 4 for fp32).

Mins (chamfer/coverage): DVE tensor_reduce(max) straight off PSUM.
Top-16 (repulsion/smoothness): ACT copies -d^2 to SBUF, then DVE
max8 -> match_replace -> max8 gives the exact 16 largest -d^2 (= 16
nearest) per row. Self-distance is forced to the reference's
sqrt(EPS)=1e-6 by overwriting rank 0.

Each core returns [128, 5] fp32 per-partition partial sums; the host
reduces and assembles the 5 scalar outputs.
"""
import numpy as np
import ml_dtypes
from contextlib import ExitStack

BF = ml_dtypes.bfloat16

B = 4
N = 4096          # pred/gt points per batch
KP = 2048         # partial points per batch
NCORES = 8
HALF_N = N // 2   # 2048 query rows per core (pred/gt)
HALF_P = KP // 2  # 1024 partial query rows per core
KAUG = 24

CHAMFER_W, REP_W, SMOOTH_W, COV_W = 1.0, 0.01, 0.005, 0.1
REP_THRESHOLD = 0.01

_NC_CACHE = {}


def _split3(x):
    h = x.astype(BF).astype(np.float32)
    m = (x - h).astype(BF).astype(np.float32)
    l = (x - h - m).astype(BF).astype(np.float32)
    return h, m, l


def _aug_query(q):
    """q [n,3] fp32 -> [24, n] bf16 lhsT rows (query side, negated norms)."""
    n = q.shape[0]
    qh, qm, ql = _split3(q)
    nq = (q * q).sum(-1)
    nqh, nqm, nql = _split3(nq)
    rows = np.zeros((KAUG, n), np.float32)
    rows[0:3] = 2 * qh.T
    rows[3:6] = 2 * qh.T
    rows[6:9] = 2 * qm.T
    rows[9:12] = 2 * qh.T
    rows[12:15] = 2 * ql.T
    rows[15:18] = 2 * qm.T
    rows[18] = -nqh
    rows[19] = -nqm
    rows[20] = -nql
    rows[21] = -1.0
    rows[22] = -1.0
    rows[23] = -1.0
    return np.ascontiguousarray(rows.astype(BF))


def _aug_db(b):
    """b [m,3] fp32 -> [24, m] bf16 rhs rows (database side)."""
    m_ = b.shape[0]
    bh, bm, bl = _split3(b)
    nb = (b * b).sum(-1)
    nbh, nbm, nbl = _split3(nb)
    rows = np.zeros((KAUG, m_), np.float32)
    rows[0:3] = bh.T
    rows[3:6] = bm.T
    rows[6:9] = bh.T
    rows[9:12] = bl.T
    rows[12:15] = bh.T
    rows[15:18] = bm.T
    rows[18] = 1.0
    rows[19] = 1.0
    rows[20] = 1.0
    rows[21] = nbh
    rows[22] = nbm
    rows[23] = nbl
    return np.ascontiguousarray(rows.astype(BF))


TOPK_FP32 = True


def _build_nc(repeat=1, n_gp=0, topk_fp32=TOPK_FP32):
    """repeat>1 wraps the body in a timing loop (benchmarking only)."""
    import concourse.bacc as bacc
    import concourse.mybir as mybir
    import concourse.tile as tile

    FP32 = mybir.dt.float32
    BF16 = mybir.dt.bfloat16
    TK = FP32 if topk_fp32 else BF16
    AX = mybir.AxisListType.X
    OP = mybir.AluOpType
    ACTF = mybir.ActivationFunctionType

    nc = bacc.Bacc("TRN2", target_bir_lowering=False, debug=False)

    qp = nc.dram_tensor("qp", [KAUG, HALF_N], BF16, kind="ExternalInput").ap()
    qg = nc.dram_tensor("qg", [KAUG, HALF_N], BF16, kind="ExternalInput").ap()
    qc = nc.dram_tensor("qc", [KAUG, HALF_P], BF16, kind="ExternalInput").ap()
    dg = nc.dram_tensor("dg", [KAUG, N], BF16, kind="ExternalInput").ap()
    dp = nc.dram_tensor("dp", [KAUG, N], BF16, kind="ExternalInput").ap()
    out = nc.dram_tensor("out", [128, 5], FP32, kind="ExternalOutput").ap()

    NT_A = HALF_N // 128   # 16 row-tiles for A (pred half vs gt)
    NT_B = HALF_N // 128   # 16 for B (gt half vs pred)
    NT_C = HALF_P // 128   # 8 for C (partial half vs pred)
    NT_D = HALF_N // 128   # 16 for D (pred half vs pred, top-16)

    with tile.TileContext(nc) as tc, ExitStack() as ctx:
        const = ctx.enter_context(tc.tile_pool(name="const", bufs=1))
        work = ctx.enter_context(tc.tile_pool(name="work", bufs=4))
        ps = ctx.enter_context(tc.tile_pool(name="ps", bufs=2, space="PSUM"))

        dps = const.tile([KAUG, N], BF16)
        qps = const.tile([KAUG, HALF_N], BF16)
        dgs = const.tile([KAUG, N], BF16)
        qgs = const.tile([KAUG, HALF_N], BF16)
        qcs = const.tile([KAUG, HALF_P], BF16)

        def load_inputs():
            # split the critical dp/qp loads across both HWDGE queues and
            # SWDGE so the first D/B/C matmuls start sooner
            nc.sync.dma_start(dps[:, 0:N // 2], dp[:, 0:N // 2])
            nc.scalar.dma_start(dps[:, N // 2:N], dp[:, N // 2:N])
            nc.gpsimd.dma_start(qps[:], qp)
            nc.sync.dma_start(dgs[:], dg)
            nc.scalar.dma_start(qgs[:], qg)
            nc.gpsimd.dma_start(qcs[:], qc)

        # chamfer maxes of -d^2: 2 columns per row-tile (one per psum half)
        mA = const.tile([128, 2 * NT_A], FP32)
        mB = const.tile([128, 2 * NT_B], FP32)
        mC = const.tile([128, 2 * NT_C], FP32)
        thr = const.tile([128, 1], FP32)      # repulsion threshold bias
        nc.gpsimd.memset(thr[:], float(REP_THRESHOLD))
        # D per-row-tile stats
        s1c = const.tile([128, NT_D], FP32)   # sum of 16 NN distances
        s2n = const.tile([128, NT_D], FP32)   # sum of -d^2 over 16 NN
        repc = const.tile([128, NT_D], FP32)  # sum relu(thr - d_{1..4})
        v16all = const.tile([128, 16 * NT_D], TK)  # top-16 -d^2 per row-tile
        S = const.tile([128, 5], FP32)        # final per-partition sums

        def mm_rowtile(q_sb, db_sb, t, consume, key):
            """8 matmuls of row-tile t (queries q_sb[:, t*128:...]) against
            db_sb, in two 4-bank PSUM tiles; consume(half, pt) eats each."""
            lhsT = q_sb[:, t * 128:(t + 1) * 128]
            for half in range(2):
                pt = ps.tile([128, 2048], FP32, tag="pt", name=f"pt_{key}_{half}")
                for j in range(4):
                    col = half * 4 + j
                    nc.tensor.matmul(
                        pt[:, j * 512:(j + 1) * 512],
                        lhsT,
                        db_sb[:, col * 512:(col + 1) * 512],
                        start=True, stop=True,
                    )
                consume(half, pt)

        def chamfer_dve(q_sb, db_sb, t, mdst, key):
            """row mins via DVE tensor_reduce straight off PSUM."""
            def consume(half, pt):
                nc.vector.tensor_reduce(
                    mdst[:, 2 * t + half:2 * t + half + 1], pt[:],
                    axis=AX, op=OP.max)
            mm_rowtile(q_sb, db_sb, t, consume, key)

        # chamfer work list: C first (offload candidates), then A, then B
        chamfer_jobs = (
            [(qcs, dps, t, mC, f"C{t}") for t in range(NT_C)]
            + [(qps, dgs, t, mA, f"A{t}") for t in range(NT_A)]
            + [(qgs, dps, t, mB, f"B{t}") for t in range(NT_B)]
        )
        routed = [(chamfer_dve, j) for j in chamfer_jobs]

        def self_mm_half(t, half, dt_):
            lhsT = qps[:, t * 128:(t + 1) * 128]
            pt = ps.tile([128, 2048], FP32, tag="pt", name=f"pt_D{t}_{half}")
            for j in range(4):
                col = half * 4 + j
                nc.tensor.matmul(
                    pt[:, j * 512:(j + 1) * 512],
                    lhsT,
                    dps[:, col * 512:(col + 1) * 512],
                    start=True, stop=True,
                )
            nc.scalar.activation(
                dt_[:, half * 2048:(half + 1) * 2048], pt[:], ACTF.Copy)

        def self_topk(t, dt_):
            v16 = v16all[:, 16 * t:16 * (t + 1)]
            nc.vector.max(v16[:, 0:8], dt_[:])
            nc.vector.match_replace(dt_[:], v16[:, 0:8], dt_[:], -1e30)
            nc.vector.max(v16[:, 8:16], dt_[:])

        def body():
            # interleave chamfer jobs between the two PSUM halves of each D
            # row-tile so psum slot handoffs alternate consumer engines
            jobs_per_d = (len(routed) + NT_D - 1) // NT_D
            k = 0
            for t in range(NT_D):
                jobs = routed[k:k + jobs_per_d]
                k += jobs_per_d
                dt_ = work.tile([128, N], TK, tag="dt", name=f"dt_{t}")
                self_mm_half(t, 0, dt_)
                if jobs:
                    fn, j = jobs[0]
                    fn(*j)
                self_mm_half(t, 1, dt_)
                for fn, j in jobs[1:]:
                    fn(*j)
                self_topk(t, dt_)

            # finish: clamp then sqrt of chamfer maxes with accumulation;
            # pair-combine the 2 psum-half columns per row-tile first
            mAf = work.tile([128, NT_A], FP32, tag="mAf")
            nc.vector.tensor_reduce(
                mAf[:], mA.rearrange("p (t two) -> p t two", two=2),
                axis=AX, op=OP.max)
            mBf = work.tile([128, NT_B], FP32, tag="mBf")
            nc.vector.tensor_reduce(
                mBf[:], mB.rearrange("p (t two) -> p t two", two=2),
                axis=AX, op=OP.max)
            mCf = work.tile([128, NT_C], FP32, tag="mCf")
            nc.vector.tensor_reduce(
                mCf[:], mC.rearrange("p (t two) -> p t two", two=2),
                axis=AX, op=OP.max)
            nc.gpsimd.tensor_scalar_min(mAf[:], mAf[:], -1e-12)
            nc.gpsimd.tensor_scalar_min(mBf[:], mBf[:], -1e-12)
            nc.gpsimd.tensor_scalar_min(mCf[:], mCf[:], -1e-12)
            dumA = work.tile([128, NT_A], FP32, tag="dumA")
            nc.scalar.activation(
                dumA[:], mAf[:], ACTF.Sqrt, scale=-1.0, accum_out=S[:, 0:1])
            dumB = work.tile([128, NT_B], FP32, tag="dumB")
            nc.scalar.activation(
                dumB[:], mBf[:], ACTF.Sqrt, scale=-1.0, accum_out=S[:, 1:2])
            dumC = work.tile([128, NT_C], FP32, tag="dumC")
            nc.scalar.activation(
                dumC[:], mCf[:], ACTF.Sqrt, scale=-1.0, accum_out=S[:, 2:3])

            # batched D finishing over all row-tiles' top-16 values:
            # force self-distance to the reference's sqrt(EPS), clamp
            # -d^2 <= -EPS (matches reference max(sq, EPS); also kills
            # positive fp noise on ultra-close pairs -> no sqrt NaN)
            v16v = v16all.rearrange("p (t k) -> p t k", k=16)
            nc.gpsimd.memset(v16v[:, :, 0:1], -1e-12)
            nc.gpsimd.tensor_scalar_min(v16all[:], v16all[:], -1e-12)
            nc.vector.tensor_reduce(s2n[:], v16v, axis=AX, op=OP.add)
            d16all = work.tile([128, 16 * NT_D], FP32, tag="d16all")
            nc.scalar.activation(
                d16all[:], v16all[:], ACTF.Sqrt, scale=-1.0)
            d16v = d16all.rearrange("p (t k) -> p t k", k=16)
            nc.vector.tensor_reduce(s1c[:], d16v, axis=AX, op=OP.add)
            rep4all = work.tile([128, 4 * NT_D], FP32, tag="rep4all")
            nc.scalar.activation(
                rep4all[:], d16v[:, :, 1:5], ACTF.Relu, scale=-1.0,
                bias=thr[:])
            nc.vector.tensor_reduce(
                repc[:], rep4all.rearrange("p (t k) -> p t k", k=4),
                axis=AX, op=OP.add)

            # 15*var per row-tile: -s2n - s1^2/16 ; accum over row-tiles
            t1 = work.tile([128, NT_D], FP32, tag="t1")
            nc.gpsimd.tensor_tensor(t1[:], s1c[:], s1c[:], op=OP.mult)
            var15 = work.tile([128, NT_D], FP32, tag="var15")
            nc.vector.scalar_tensor_tensor(
                var15[:], t1[:], -1.0 / 16.0, s2n[:],
                op0=OP.mult, op1=OP.subtract,
                accum_out=S[:, 3:4])
            nc.vector.tensor_reduce(S[:, 4:5], repc[:], axis=AX, op=OP.add)

        if repeat == 1:
            load_inputs()
            body()
        else:
            # input DMAs live inside the loop so no dependency crosses the
            # back-edge semaphore reset
            with tc.For_i(0, repeat, 1):
                load_inputs()
                body()

        nc.gpsimd.dma_start(out, S[:])

    nc.compile()
    return nc


def _get_nc():
    if "nc" not in _NC_CACHE:
        _NC_CACHE["nc"] = _build_nc()
    return _NC_CACHE["nc"]


def _make_in_maps(pred, gt, partial):
    in_maps = []
    dbg = [_aug_db(gt[b]) for b in range(B)]
    dbp = [_aug_db(pred[b]) for b in range(B)]
    for c in range(NCORES):
        b, h = divmod(c, 2)
        in_maps.append({
            "qp": _aug_query(pred[b, h * HALF_N:(h + 1) * HALF_N]),
            "qg": _aug_query(gt[b, h * HALF_N:(h + 1) * HALF_N]),
            "qc": _aug_query(partial[b, h * HALF_P:(h + 1) * HALF_P]),
            "dg": dbg[b],
            "dp": dbp[b],
        })
    return in_maps


def _combine(results):
    S = np.stack([r["out"] for r in results]).astype(np.float64)  # [8,128,5]
    tot = S.sum(axis=(0, 1))
    cd = tot[0] / (B * N) + tot[1] / (B * N)
    cov = tot[2] / (B * KP)
    smooth = tot[3] / 15.0 / (B * N)
    rep = tot[4] / (B * N * 4)
    total = (CHAMFER_W * cd + REP_W * rep + SMOOTH_W * smooth + COV_W * cov)
    return tuple(np.float32(x) for x in (total, cd, rep, smooth, cov))


def _get_runner():
    """Cached jitted SPMD executor (mirrors bass2jax.run_bass_via_pjrt but
    reuses the traced/jitted callable across kernel() calls)."""
    if "runner" in _NC_CACHE:
        return _NC_CACHE["runner"]
    import jax
    import concourse.mybir as mybir
    from concourse import bass2jax
    from jax.experimental.shard_map import shard_map
    from jax.sharding import Mesh, PartitionSpec

    nc = _get_nc()
    bass2jax.install_neuronx_cc_hook()
    assert nc.dbg_addr is None
    pname = nc.partition_id_tensor.name if nc.partition_id_tensor else None

    in_names, out_names, out_avals, zero_outs = [], [], [], []
    for alloc in nc.m.functions[0].allocations:
        if not isinstance(alloc, mybir.MemoryLocationSet):
            continue
        name = alloc.memorylocations[0].name
        if alloc.kind == "ExternalInput":
            if name != pname:
                in_names.append(name)
        elif alloc.kind == "ExternalOutput":
            shape = tuple(alloc.tensor_shape)
            dtype = mybir.dt.np(alloc.dtype)
            out_names.append(name)
            out_avals.append(jax.core.ShapedArray(shape, dtype))
            zero_outs.append(np.zeros((NCORES * shape[0], *shape[1:]), dtype))
    n_params = len(in_names)
    all_in_names = in_names + out_names
    if pname is not None:
        all_in_names = all_in_names + [pname]
    donate = tuple(range(n_params, n_params + len(out_names)))

    def _body(*args):
        operands = list(args)
        if pname is not None:
            operands.append(bass2jax.partition_id_tensor())
        outs = bass2jax._bass_exec_p.bind(
            *operands,
            out_avals=tuple(out_avals),
            in_names=tuple(all_in_names),
            out_names=tuple(out_names),
            lowering_input_output_aliases=(),
            sim_require_finite=True,
            sim_require_nnan=True,
            nc=nc,
        )
        return tuple(outs)

    devices = jax.devices()[:NCORES]
    mesh = Mesh(np.asarray(devices), ("core",))
    nio = n_params + len(out_names)
    sharded = jax.jit(
        shard_map(
            _body, mesh=mesh,
            in_specs=(PartitionSpec("core"),) * nio,
            out_specs=(PartitionSpec("core"),) * len(out_names),
            check_rep=False,
        ),
        donate_argnums=donate,
        keep_unused=True,
    )

    def run(in_maps):
        concat_in = [
            np.concatenate([m[name] for m in in_maps], axis=0)
            for name in in_names
        ]
        out_arrs = sharded(*concat_in, *[z.copy() for z in zero_outs])
        return [
            {
                name: np.asarray(out_arrs[i]).reshape(
                    NCORES, *out_avals[i].shape)[c]
                for i, name in enumerate(out_names)
            }
            for c in range(NCORES)
        ]

    _NC_CACHE["runner"] = run
    return run


def kernel(pred, gt, partial):
    pred = np.asarray(pred, dtype=np.float32)
    gt = np.asarray(gt, dtype=np.float32)
    partial = np.asarray(partial, dtype=np.float32)

    run = _get_runner()
    in_maps = _make_in_maps(pred, gt, partial)
    return _combine(run(in_maps))
